# revision 18
# baseline (speedup 1.0000x reference)
"""CARAFE++ (content-aware reassembly upsampling) Trainium2 kernel.

Full inputs in / full outputs out; internally sharded data-parallel over 8
NeuronCores: core = (sample n = core//2, h-half hh = core%2).

Per-core pipeline (padded coords: 36 rows x 68 cols around the 32x64 slice):
  1. conv1x1 (256->64, output duplicated onto both psum partition halves) on
     PE, fp16 operands.
  2. conv3x3 (64->100) as 6 PSUM-accumulated matmuls per 8-row chunk using a
     partition-packed (dx=0,1) copy of `compressed`.
  3. exp on ACT (softmax numerator; b_enc folded in as the activation bias);
     per-q sums via a selector matmul on PE.  Normalization (divide by the
     sum) happens on the HOST: the device emits the *unnormalized*
     reassembly plus the tiny sums tensor.
  4. CARAFE reassembly: per 4-pixel group a block-diagonal weight matrix
     Wbig[100,16] (4 blocks of [25 taps x 4 subpixels]) multiplies a
     gathered patch operand RHS[100, 256ch] (im2col via SBUF->SBUF DMAs from
     a resident transposed-x tile).  Four groups run concurrently in the PE
     array via column tiling (tile_position=(0,32j), m=16 each).
"""

import sys

sys.path.insert(0, "/opt/trn_rl_repo")

import numpy as np

import concourse.bass as bass
import concourse.mybir as mybir
import concourse.bacc as bacc
import concourse.tile as tile
from concourse.ap import AP
from concourse.bass_utils import run_bass_kernel_spmd

F16 = mybir.dt.float16
F32 = mybir.dt.float32

R, U = 36, 68          # padded rows/cols per core slice
PIX = R * U            # 2448
H, W = 32, 64          # output coarse rows (per core) / cols

_CACHE = {}


def _build(dbg=False):
    nc = bacc.Bacc("TRN2", target_bir_lowering=False, debug=False)

    xs16 = nc.declare_dram_parameter("xs16", [256, PIX], F16, isOutput=False)
    xt16 = nc.declare_dram_parameter("xt16", [PIX, 256], F16, isOutput=False)
    wc = nc.declare_dram_parameter("wc", [2, 128, 128], F16, isOutput=False)
    wep = nc.declare_dram_parameter("wep", [3, 128, 100], F16, isOutput=False)
    wes = nc.declare_dram_parameter("wes", [3, 64, 100], F16, isOutput=False)
    sel = nc.declare_dram_parameter("sel", [100, 4], F16, isOutput=False)
    be = nc.declare_dram_parameter("be", [100, 1], F32, isOutput=False)
    bc = nc.declare_dram_parameter("bc", [128, 1], F32, isOutput=False)
    # o[h, j, g', q, pass, c]: pixel w = 16*pass + 4*j + g', subpixel q
    o = nc.declare_dram_parameter("o", [H, 4, 4, 4, 4, 256], F32, isOutput=True)
    osum = nc.declare_dram_parameter("osum", [4, H * W], F32, isOutput=True)
    if dbg:
        dbg_exp3 = nc.declare_dram_parameter("dbg_exp3", [25, H * W * 4], F16,
                                             isOutput=True)
        dbg_wbig = nc.declare_dram_parameter("dbg_wbig", [128, 2048], F16,
                                             isOutput=True)
        dbg_rhs = nc.declare_dram_parameter("dbg_rhs", [128, 4096], F16,
                                            isOutput=True)

    with tile.TileContext(nc) as tc:
        with (
            tc.tile_pool(name="consts", bufs=1) as cpool,
            tc.tile_pool(name="data", bufs=1) as dpool,
            tc.tile_pool(name="wbig", bufs=2) as wpool,
            tc.tile_pool(name="rhs", bufs=3) as rpool,
            tc.tile_pool(name="outp", bufs=3) as opool,
            tc.tile_pool(name="cp_ps", bufs=1, space="PSUM") as cp_pool,
            tc.tile_pool(name="lg_ps", bufs=1, space="PSUM") as lg_pool,
            tc.tile_pool(name="sm_ps", bufs=1, space="PSUM") as sm_pool,
            tc.tile_pool(name="o_ps", bufs=2, space="PSUM") as o_pool,
        ):
            # ---- constants / inputs into SBUF ----
            wc_sb = cpool.tile([128, 256], F16, tag="wc")
            nc.sync.dma_start(wc_sb[:].rearrange("p (j m) -> p j m", j=2),
                              wc.ap().rearrange("j p m -> p j m"))
            wep_sb = cpool.tile([128, 300], F16, tag="wep")
            nc.sync.dma_start(wep_sb[:].rearrange("p (d m) -> p d m", d=3),
                              wep.ap().rearrange("d p m -> p d m"))
            wes_sb = cpool.tile([64, 300], F16, tag="wes")
            nc.sync.dma_start(wes_sb[:].rearrange("p (d m) -> p d m", d=3),
                              wes.ap().rearrange("d p m -> p d m"))
            sel_sb = cpool.tile([100, 4], F16, tag="sel")
            nc.sync.dma_start(sel_sb[:], sel.ap())
            be_sb = cpool.tile([100, 1], F32, tag="be")
            nc.sync.dma_start(be_sb[:], be.ap())
            bc_sb = cpool.tile([128, 1], F32, tag="bc")
            nc.sync.dma_start(bc_sb[:], bc.ap())

            xs_sb = dpool.tile([128, 2 * PIX], F16, tag="xs")
            nc.sync.dma_start(xs_sb[:].rearrange("p (j f) -> p j f", j=2),
                              xs16.ap().rearrange("(j p) f -> p j f", j=2))

            comp2_sb = dpool.tile([128, PIX], F16, tag="comp2")
            exp_sb = dpool.tile([100, H * W], F16, tag="exp")
            # exp3[kk, h*256 + w*4 + q]: q moved to the free dim so the
            # block-diagonal Wbig builds are rectangular DMAs
            exp3_sb = dpool.tile([25, H * W * 4], F16, tag="exp3")
            sums_sb = dpool.tile([4, H * W], F32, tag="sums")

            # ---- conv1x1: compressed (duplicated on both partition halves) --
            chunks = [(0, 512), (512, 512), (1024, 512), (1536, 512), (2048, 400)]
            for s, w in chunks:
                ps = cp_pool.tile([128, 512], F32, tag="cp")
                nc.tensor.matmul(ps[:, :w], wc_sb[:, 0:128], xs_sb[:, s:s + w],
                                 start=True, stop=False)
                nc.tensor.matmul(ps[:, :w], wc_sb[:, 128:256],
                                 xs_sb[:, PIX + s:PIX + s + w],
                                 start=False, stop=True)
                # upper half: comp[ch, f]; lower half: comp[ch, f+1] (dx-shift)
                nc.vector.tensor_scalar_add(comp2_sb[0:64, s:s + w], ps[0:64, :w],
                                            bc_sb[0:64, :])
                if s == 0:
                    nc.vector.tensor_scalar_add(comp2_sb[64:128, 0:w - 1],
                                                ps[64:128, 1:w], bc_sb[64:128, :])
                else:
                    nc.vector.tensor_scalar_add(comp2_sb[64:128, s - 1:s + w - 1],
                                                ps[64:128, :w], bc_sb[64:128, :])

            comp3 = comp2_sb[:].rearrange("p (r u) -> p r u", u=U)

            # ---- conv3x3 -> logits -> exp -> sums, per 8-row chunk ----
            for hb in range(4):
                h0 = hb * 8
                lg = lg_pool.tile([100, 512], F32, tag="lg")
                for dy in range(3):
                    r0 = h0 + 1 + dy
                    nc.tensor.matmul(lg[:, :],
                                     wep_sb[:, dy * 100:(dy + 1) * 100],
                                     comp3[0:128, r0:r0 + 8, 1:65],
                                     start=(dy == 0), stop=False)
                    nc.tensor.matmul(lg[:, :],
                                     wes_sb[:, dy * 100:(dy + 1) * 100],
                                     comp3[0:64, r0:r0 + 8, 3:67],
                                     start=False, stop=(dy == 2))
                nc.scalar.activation(exp_sb[:, h0 * 64:h0 * 64 + 512], lg[:, :],
                                     mybir.ActivationFunctionType.Exp,
                                     bias=be_sb[:], scale=1.0)
                sm = sm_pool.tile([4, 512], F32, tag="sm")
                nc.tensor.matmul(sm[:, :], sel_sb[:, :],
                                 exp_sb[:, h0 * 64:h0 * 64 + 512],
                                 start=True, stop=True)
                nc.vector.tensor_copy(sums_sb[:, h0 * 64:h0 * 64 + 512], sm[:, :])

                # exp3: move q from partitions to the free dim
                for q in range(4):
                    nc.sync.dma_start(
                        exp3_sb[0:25, :].rearrange(
                            "p (h w qq) -> p h w qq", h=H, w=W, qq=4
                        )[:, h0:h0 + 8, :, q],
                        exp_sb[:, :].rearrange(
                            "(kk qq) (h w) -> qq kk h w", qq=4, w=W
                        )[q, :, h0:h0 + 8, :],
                    )

            nc.sync.dma_start(osum.ap(), sums_sb[:])

            # ---- reassembly: block-diag Wbig + gathered patches ----
            exp3v = exp3_sb[0:25, :].rearrange("p (hg x) -> p hg x", x=16)
            o_v = o.ap()

            for hb in range(4):
                h0 = hb * 8
                wbig = wpool.tile([128, 8 * 16 * 16], F16, tag="wbig")
                # zero the off-block entries once per tile buffer: blocks are
                # disjoint, every (partition, free) slot not covered by some
                # g' block must be 0.  memset the whole tile, DMA values in.
                nc.vector.memset(wbig[:], 0.0)
                for gp in range(4):
                    # dst[25*gp + kk, gg*16 + 4*gp + q] = exp3[kk, (h,g)*16+4gp+q]
                    dst = wbig[25 * gp:25 * gp + 25, :].rearrange(
                        "p (gg x) -> p gg x", x=16)[:, :, 4 * gp:4 * gp + 4]
                    src = exp3v[:, h0 * 16:h0 * 16 + 128,
                                4 * gp:4 * gp + 4]
                    nc.sync.dma_start(dst, src)

                for h in range(8):
                    habs = h0 + h
                    rhs = rpool.tile([128, 16 * 256], F16, tag="rhs")
                    for ki in range(5):
                        for gp in range(4):
                            # dst partitions 25g'+5ki+dj, free g*256+c;
                            # src xt16[(habs+ki)*68 + 4g+g'+dj, c] (flat DRAM)
                            dstv = rhs[25 * gp + 5 * ki:25 * gp + 5 * ki + 5, :] \
                                .rearrange("p (g c) -> p g c", c=256)
                            srcv = AP(
                                xt16,
                                ((habs + ki) * U + gp) * 256,
                                [[256, 5], [1024, 16], [1, 256]],
                            )
                            eng = nc.sync if (ki + gp) % 4 != 3 else nc.scalar
                            eng.dma_start(dstv, srcv)
                    osb = opool.tile([128, 4 * 256], F32, tag="osb")
                    ops = o_pool.tile([128, 1024], F32, tag="ops")
                    for ps_i in range(4):
                        for j in range(4):
                            g = 4 * ps_i + j
                            nc.tensor.matmul(
                                ops[32 * j:32 * j + 16,
                                    ps_i * 256:(ps_i + 1) * 256],
                                wbig[0:100,
                                     (h * 16 + g) * 16:(h * 16 + g) * 16 + 16],
                                rhs[0:100, g * 256:(g + 1) * 256],
                                start=True, stop=True,
                                tile_position=(0, 32 * j),
                            )
                    if h % 2 == 0:
                        nc.vector.tensor_copy(osb[:], ops[:])
                    else:
                        nc.scalar.activation(osb[:], ops[:],
                                             mybir.ActivationFunctionType.Copy)
                    # o[habs, j, (g',q)=m, (pass,c)] <- osb[32j+m, pass*256+c]
                    for j in range(4):
                        dsto = AP(o, habs * 65536 + j * 16384,
                                  [[1024, 16], [1, 1024]])
                        nc.sync.dma_start(dsto, osb[32 * j:32 * j + 16, :])
                    if dbg and habs == 0:
                        nc.sync.dma_start(dbg_rhs.ap(), rhs[:])
                if dbg and hb == 0:
                    nc.sync.dma_start(dbg_wbig.ap(), wbig[:])
            if dbg:
                nc.sync.dma_start(dbg_exp3.ap(), exp3_sb[:])

    nc.compile()
    return nc


def _get_nc():
    if "nc" not in _CACHE:
        _CACHE["nc"] = _build()
    return _CACHE["nc"]


def _prep_inputs(x, w_comp, b_comp, w_enc, b_enc):
    xp = np.zeros((4, 256, 68, 68), np.float32)
    xp[:, :, 2:66, 2:66] = x

    wcm = w_comp.reshape(64, 256).T                      # [256(in), 64(out)]
    wc_dup = np.ascontiguousarray(
        np.tile(wcm.reshape(2, 128, 64), (1, 1, 2))).astype(np.float16)
    wepm = np.ascontiguousarray(np.stack(
        [np.concatenate([w_enc[:, :, dy, 0].T, w_enc[:, :, dy, 1].T], axis=0)
         for dy in range(3)])).astype(np.float16)        # [3, 128, 100]
    wesm = np.ascontiguousarray(np.stack(
        [w_enc[:, :, dy, 2].T for dy in range(3)])).astype(np.float16)
    selm = np.zeros((100, 4), np.float16)
    selm[np.arange(100), np.arange(100) % 4] = 1.0
    bem = b_enc.reshape(100, 1).astype(np.float32)
    bcm = np.concatenate([b_comp, b_comp]).reshape(128, 1).astype(np.float32)

    in_maps = []
    for core in range(8):
        n, hh = core >> 1, core & 1
        sl = xp[n, :, 32 * hh:32 * hh + 36, :]           # [256, 36, 68]
        xs = np.ascontiguousarray(sl.reshape(256, PIX)).astype(np.float16)
        xt = np.ascontiguousarray(sl.reshape(256, PIX).T).astype(np.float16)
        in_maps.append({
            "xs16": xs, "xt16": xt, "wc": wc_dup, "wep": wepm, "wes": wesm,
            "sel": selm, "be": bem, "bc": bcm,
        })
    return in_maps


def kernel(x, w_comp, b_comp, w_enc, b_enc):
    x = np.asarray(x, np.float32)
    w_comp = np.asarray(w_comp, np.float32)
    b_comp = np.asarray(b_comp, np.float32)
    w_enc = np.asarray(w_enc, np.float32)
    b_enc = np.asarray(b_enc, np.float32)

    nc = _get_nc()
    in_maps = _prep_inputs(x, w_comp, b_comp, w_enc, b_enc)
    res = run_bass_kernel_spmd(nc, in_maps, core_ids=list(range(8)))

    out = np.empty((4, 256, 128, 128), np.float32)
    for core in range(8):
        n, hh = core >> 1, core & 1
        ob = res.results[core]["o"].reshape(H, 4, 4, 4, 4, 256)
        osum = res.results[core]["osum"].reshape(4, H, W)     # [q, h, w]
        # pixel w = 16*pass + 4*j + g'  -> ob[h, j, g', q, pass, c]
        o2 = ob.transpose(0, 4, 1, 2, 3, 5)                   # [h,pass,j,g',q,c]
        o2 = o2.reshape(H, W, 4, 256)                         # [h, w, q, c]
        s = osum.transpose(1, 2, 0)[:, :, :, None]            # [h, w, q, 1]
        o2 = o2 / s
        # q = 2*sr + sc ; out rows 2*(32hh+h)+sr, cols 2w+sc
        o2 = o2.reshape(H, W, 2, 2, 256)                      # [h, w, sr, sc, c]
        blk = o2.transpose(4, 0, 2, 1, 3).reshape(256, 64, 128)
        out[n, :, 64 * hh:64 * hh + 64, :] = blk
    return out
